# revision 2
# baseline (speedup 1.0000x reference)
"""Trainium2 Bass kernel for nn_BioConvolution — single-stream DMA redesign.

Same math/precision as the baseline kernel (fp8 e3m4 filters x256, patches
half e3m4 / half f16 x2, measured rel err 1.81e-2), but the dataflow is
rebuilt around two measured HW facts:
  1. DMA bandwidth on this part is maximized by ONE engine ring issuing few
     LARGE-descriptor DMAs over one sequential address stream (~940 GB/s
     for a 56KB/partition stream vs ~525 GB/s for two 28KB rings and
     ~300-600 GB/s for any SWDGE mix).  Concurrent rings CONTEND.
  2. The PE is not weight-load-bound: 64x(128->64) matmuls with fresh
     stationary weights run at the 64-row moving roofline (~1.5 us), so
     256 matmuls/iter ~ 6.8 us is the compute floor.

Dataflow per core per iteration:
  - ALL inputs are host-packed into ONE DRAM tensor pk [128, 57344] u8:
    per column c (16 of them): [fl_c 2048B | xs16_c 1024B | xs8_c 512B],
    i.e. filters (r,kk,f) fp8, patches-f16 (kk',p) and patches-fp8 (kk,p)
    with p = r*64+n, all pre-transposed K-major on the host.
  - sync ring streams pk in 4 chunks of 4 columns (14336B/partition
    descriptors) — the single input stream.
  - Per chunk: 64 matmuls (4 cols x 2 locs x 8 k-tiles) accumulate into
    ONE PSUM bank tile [128, 512]; ONE scalar activation applies
    relu(x/512 + bias) for all 8 locations at once (vs 8 separate acts);
    one scalar-ring DMA stores the f16 [F, 512] chunk.
No SWDGE, no gathers; gpsimd and vector engines idle.
"""
import numpy as np
import ml_dtypes

N, H, W, C = 64, 64, 64, 64
FH, FW, F = 4, 4, 128
R = Cc = 16          # 16x16 patch grid
K = FH * FW * C      # 1024 contraction
KK = K // 128        # 8 k-tiles of 128
NC_CORES = 8
RPC = R // NC_CORES  # patch rows per core = 2
W_SCALE = 256.0      # filters pre-scale into e3m4 normal range
X_SCALE = 2.0        # patches pre-scale (exact in f16; keeps fp8 in range)
S = 4                # patch k-groups stored in fp8 (of KK=8)

GCOL = 4             # columns per pipeline chunk
CSTRIDE = RPC * KK * F + (KK - S) * 128 * 2 + S * 128  # 3584 B/column
FL_OFF = 0
X16_OFF = RPC * KK * F                 # 2048
X8_OFF = X16_OFF + (KK - S) * 128 * 2  # 3072

_compiled = {}


def _host_shards(X, filters, bias, dtype=np.float16):
    """Pack each core's inputs into one [128, 16*3584] byte tensor."""
    X = np.asarray(X, np.float32)
    filters = np.asarray(filters, np.float32)
    bias = np.asarray(bias, np.float32)
    f8 = ml_dtypes.float8_e3m4

    # B[r, n, c, K]: patch row r, batch n, column c, K = (i*4+j)*64+ch
    A = X.reshape(N, R, FH, Cc, FW, C)                     # n r i c j ch
    B = np.ascontiguousarray(A.transpose(1, 0, 3, 2, 4, 5)).reshape(R, N, Cc, K)
    # filters q-major per core: fl[q, c, r_local, kk, f]
    flt = filters[0].reshape(8, RPC, Cc, KK, 128, F)       # a r c kk q f
    fl9 = flt.transpose(0, 4, 2, 1, 3, 5)                  # a q c r kk f
    fl9 = np.clip(fl9 * W_SCALE, -15.5, 15.5).astype(f8)

    in_maps = []
    for a in range(NC_CORES):
        Bc = B[2 * a: 2 * a + 2].reshape(RPC, N, Cc, KK, 128)  # r n c kk q
        xsT = Bc.transpose(4, 2, 3, 0, 1) * X_SCALE            # q c kk r n
        xs8 = np.clip(xsT[:, :, :S], -15.5, 15.5).astype(f8)   # q c S r n
        xs16 = xsT[:, :, S:].astype(np.float16)                # q c S' r n
        pk = np.concatenate([
            np.ascontiguousarray(fl9[a]).view(np.uint8).reshape(128, Cc, -1),
            np.ascontiguousarray(xs16).view(np.uint8).reshape(128, Cc, -1),
            np.ascontiguousarray(xs8).view(np.uint8).reshape(128, Cc, -1),
        ], axis=2)
        assert pk.shape == (128, Cc, CSTRIDE), pk.shape
        in_maps.append({
            "pk": np.ascontiguousarray(pk.reshape(128, Cc * CSTRIDE)),
            "bias": bias.reshape(F, 1),
        })
    return in_maps


def _build(n_iters=1):
    import concourse.mybir as mybir
    import concourse.tile as tile
    from concourse import bacc

    f8 = mybir.dt.float8e3
    f16 = mybir.dt.float16
    u8 = mybir.dt.uint8
    nc = bacc.Bacc("TRN2", target_bir_lowering=False, debug=False,
                   num_devices=NC_CORES)
    pk_d = nc.dram_tensor("pk", [128, Cc * CSTRIDE], u8,
                          kind="ExternalInput").ap()
    bias_d = nc.dram_tensor("bias", [F, 1], mybir.dt.float32,
                            kind="ExternalInput").ap()
    out_d = nc.dram_tensor("out", [F, Cc * RPC * N], f16,
                           kind="ExternalOutput").ap()
    relu = mybir.ActivationFunctionType.Relu
    NG = Cc // GCOL                      # 4 chunks
    CHUNK = GCOL * CSTRIDE               # 14336 B/partition

    with tile.TileContext(nc) as tc:
        with (
            tc.tile_pool(name="const", bufs=1) as const_pool,
            tc.tile_pool(name="pk", bufs=2) as pk_pool,
            tc.tile_pool(name="ps", bufs=4, space="PSUM") as ps_pool,
            tc.tile_pool(name="og", bufs=3) as og_pool,
        ):
            bias_t = const_pool.tile([F, 1], mybir.dt.float32, tag="bias")
            nc.sync.dma_start(bias_t[:], bias_d[:])

            for _ in range(n_iters):
                for g in range(NG):
                    t = pk_pool.tile([128, CHUNK], u8, tag="pk")
                    nc.sync.dma_start(t[:], pk_d[:, g * CHUNK:(g + 1) * CHUNK])
                    ps = ps_pool.tile([128, GCOL * RPC * N],
                                      mybir.dt.float32, tag="ps")
                    for ci in range(GCOL):
                        base = ci * CSTRIDE
                        for r in range(RPC):
                            col = ci * RPC + r
                            for k in range(KK):
                                lhsT = t[:, base + (r * KK + k) * F
                                         : base + (r * KK + k + 1) * F
                                         ].bitcast(f8)
                                if k < S:
                                    o = base + X8_OFF + k * 128 + r * N
                                    rhs = t[:, o:o + N].bitcast(f8)
                                else:
                                    o = base + X16_OFF + (k - S) * 256 + r * 128
                                    rhs = t[:, o:o + 128].bitcast(f16)
                                nc.tensor.matmul(
                                    ps[:, col * N:(col + 1) * N],
                                    lhsT=lhsT, rhs=rhs,
                                    start=(k == 0), stop=(k == KK - 1))
                    og = og_pool.tile([F, GCOL * RPC * N], f16, tag="og")
                    nc.scalar.activation(
                        og[:], ps[:], relu, bias=bias_t[:],
                        scale=1.0 / (W_SCALE * X_SCALE))
                    nc.scalar.dma_start(
                        out_d[:, g * GCOL * RPC * N:(g + 1) * GCOL * RPC * N],
                        og[:])
    nc.compile()
    return nc


def kernel(X, filters, bias):
    from concourse.bass_utils import run_bass_kernel_spmd

    assert X.shape == (N, H, W, C), X.shape
    assert filters.shape == (1, R * Cc, FH, FW, C, F), filters.shape
    assert bias.shape == (F,), bias.shape

    in_maps = _host_shards(X, filters, bias)
    if "nc" not in _compiled:
        _compiled["nc"] = _build(n_iters=1)
    res = run_bass_kernel_spmd(_compiled["nc"], in_maps, list(range(NC_CORES)))

    # res[a]["out"]: [F, Cc, RPC, N] -> [N, RPC, Cc, F] per core
    shards = [np.asarray(res.results[a]["out"], np.float32)
              .reshape(F, Cc, RPC, N).transpose(3, 2, 1, 0)
              for a in range(NC_CORES)]
    out = np.stack(shards, axis=1)             # [N, 8, RPC, Cc, F]
    return np.ascontiguousarray(out.reshape(N, R, Cc, F)).astype(np.float32)
